# revision 23
# baseline (speedup 1.0000x reference)
"""Trainium2 Bass kernel for causal cosine-sim multi-head attention.

Reference computation (per batch b of 4, 2048 tokens, dim 1024):
  q,k,v = x @ Wq, x @ Wk, x @ Wv          (inner = 8 heads x 64)
  q,k l2-normalized per head, scale 8.0, causal softmax, out = attn-out @ Wo

Sharding: 8 cores = 4 batches x 2 head-groups (4 heads each).
Core c handles batch c//2, heads [4*(c%2), 4*(c%2)+4).  Each core computes a
partial output (2048, 1024) = o_g @ Wo_g; host sums the two head-group
partials per batch.  No on-chip collectives; the 8 cores run SPMD.

Schedule notes (final):
- Software pipeline with attention lagging one i-tile behind the
  projections: iteration i runs qkv_proj(i) + norms(i) on a dense PE
  stream, then attn(i-1).  The tile-i rsqrt (ACT) therefore only competes
  with the long-drained exp backlog of attn(i-2), never stalling the
  norm->bc->S chain, and the PE never sees an ACT-FIFO bubble.
- qts/kts are 128-partition with zero rows 64..127: the S stationary is
  128-deep, so FWL hides its LDWEIGHTS (64-deep stationaries measured
  +156 ns/matmul of exposed weight-load; this was worth ~20 us).
- All ACT work (softmax exp + the l2-norm rsqrt = exp(-0.5*ln(x))) lives in
  the single natural_log_exp table set: the activation-table map is patched
  (compile-time only, index-preserving) so the table pass can only pick that
  set -> zero ACT_TABLE_LOADs in steady state.
- A ~4us dummy-matmul "heater" at kernel start pre-warms the PE_HAM clock
  gate (K=4/8 -> 8/8) under the input-DMA window.
- O^T transpose runs on the DMA XBAR (dma_start_transpose), not the PE, so
  the matmul queue never stalls behind the DVE normalize chain; the last
  tile uses the PE/identity transpose instead, keeping the ~1.2us XBAR
  latency off the critical tail.
- GpSimd (otherwise idle) takes only same-type TensorTensor multiplies (the
  causal mask apply + the q/k squares) -> no Q7 library reloads, and no
  cross-iteration Pool-FIFO priority inversions.
- Diagonal S/exp blocks are issued first within each pair (jco order) so
  their mask multiplies are ready before the AV accumulation phase.
- Softmax 1/sum comes free from the [V | 1] ones column in the AV matmul.
- Causal diagonal blocks compute only the needed column ranges in S, exp, AV.
"""

import numpy as np

import concourse.bass as bass
import concourse.bacc as bacc
import concourse.mybir as mybir
import concourse.tile as tile
import concourse.hw_specs as _hw_specs
from concourse.bass_utils import run_bass_kernel_spmd

DT = mybir.dt
F32 = DT.float32
BF16 = DT.bfloat16

N_TOK = 2048
DIM = 1024
DG = 256          # inner dims per core (4 heads x 64)
NH = 4            # heads per core
DH = 64
MOUT = 1024

_KEEP_TABLE = "natural_log_exp_and_others"


def _patch_act_tables():
    """Make the compile-time table pass see only one usable ACT table set
    (ln+exp+copy+...), so Ln and Exp never thrash table loads.  Names/indices
    are preserved (act_func_set_id is positional); only the advertised
    contents change, and the kernel only uses funcs the real set contains."""
    import functools

    orig = _hw_specs.get_activation_tables
    if getattr(orig, "_one_set_patch", False):
        return
    inner = getattr(orig, "__wrapped__", orig)

    @functools.cache
    def patched(arch):
        t = inner(arch)
        assert _KEEP_TABLE in t, sorted(t)
        return {name: (set(funcs) if name == _KEEP_TABLE else set())
                for name, funcs in t.items()}

    patched._one_set_patch = True
    _hw_specs.get_activation_tables = patched
    bacc.get_activation_tables = patched


def build_nc(N=N_TOK):
    _patch_act_tables()
    NKC = DIM // 128          # 8 contraction chunks
    NTC = N // 128            # token chunks
    QT = 512                  # token tile (qkv projection and attention i)
    NQT = N // QT
    AF = mybir.ActivationFunctionType

    nc = bacc.Bacc("TRN2", target_bir_lowering=False, debug=False, num_devices=8)
    xt_ext = nc.dram_tensor("xt", [DIM, N], BF16, kind="ExternalInput")
    wq_ext = nc.dram_tensor("wq", [128, NKC, DG], BF16, kind="ExternalInput")
    wk_ext = nc.dram_tensor("wk", [128, NKC, DG], BF16, kind="ExternalInput")
    wv_ext = nc.dram_tensor("wv", [128, NKC, DG], BF16, kind="ExternalInput")
    wo_ext = nc.dram_tensor("wo", [128, DG // 128, MOUT], BF16,
                            kind="ExternalInput")
    idm_ext = nc.dram_tensor("idm", [128, 128], BF16, kind="ExternalInput")
    out_ext = nc.dram_tensor("out", [MOUT, N], BF16, kind="ExternalOutput")

    with tile.TileContext(nc) as tc:
        with (
            tc.tile_pool(name="persist", bufs=1) as pp,
            tc.tile_pool(name="stage", bufs=3) as st,
            tc.tile_pool(name="attn_sb", bufs=4) as asb,
            tc.tile_pool(name="ps_mm", bufs=2, space="PSUM") as psM,
            tc.tile_pool(name="ps_s", bufs=2, space="PSUM") as psS,
            tc.tile_pool(name="ps_o", bufs=1, space="PSUM") as psO,
        ):
            xt = pp.tile([128, NKC, N], BF16, tag="xt")          # x transposed
            wq_sb = pp.tile([128, NKC, DG], BF16, tag="wq")
            wk_sb = pp.tile([128, NKC, DG], BF16, tag="wk")
            wv_sb = pp.tile([128, NKC, DG], BF16, tag="wv")
            wo_sb = pp.tile([128, 2, MOUT], BF16, tag="wo")
            # 128-partition q/k with zero rows 64-127: the S stationary is
            # then 128-deep -> FWL hides its LDWEIGHTS (64-deep stationaries
            # measured +156ns/matmul of exposed weight-load)
            qts = pp.tile([128, NH, N], BF16, tag="qts")         # scaled Q^T
            kts = pp.tile([128, NH, N], BF16, tag="kts")         # scaled K^T
            vt = pp.tile([128, NTC, NH, DH + 1], BF16, tag="vt")  # [V | 1]
            ot = pp.tile([128, 2, N], BF16, tag="ot")            # normed O^T
            hmat = pp.tile([128, 33], BF16, tag="hmat")  # head-sum matrix
            # broadcast selector: out rows 0..63 <- rhs row 0 (even head),
            # rows 64..127 <- rhs row 32 (odd head); one matmul per pair
            sel33 = pp.tile([33, 128], BF16, tag="sel33")
            # causal mask for the partial 128 columns of a diagonal block,
            # duplicated for the 2 heads of a pair: keep where col >= row
            mask2 = pp.tile([128, 2, 128], BF16, tag="mask2")
            ones2 = pp.tile([128, 2, 128], BF16, tag="ones2")
            idm = pp.tile([128, 128], BF16, tag="idm")   # identity (transpose)

            # ---- input DMAs first.  First Q-projection gates on wq-dc0 + the
            # 8 k-chunks of tile 0, so those go first, tile-0 chunks spread
            # across all three DMA issue queues (sync/scalar/gpsimd) ----
            xv = xt_ext.rearrange("(c p) n -> p c n", p=128)
            # per-queue, ordered by first PE use: wq-dc0, x-tile0, wq-dc1,
            # wk (K-proj at ~PE+2us), wv (V at ~PE+6us), x tiles 1-3, wo
            nc.gpsimd.dma_start(wq_sb[:, :, 0:128], wq_ext[:, :, 0:128])
            t0q = (nc.sync, nc.scalar, nc.gpsimd)
            for kc in range(NKC):
                t0q[kc % 3].dma_start(xt[:, kc:kc + 1, 0:QT],
                                      xv[:, kc:kc + 1, 0:QT])
            nc.gpsimd.dma_start(wq_sb[:, :, 128:DG], wq_ext[:, :, 128:DG])
            nc.gpsimd.dma_start(wk_sb[:, :, 0:128], wk_ext[:, :, 0:128])
            nc.sync.dma_start(wk_sb[:, :, 128:DG], wk_ext[:, :, 128:DG])
            nc.scalar.dma_start(wv_sb[:, :, 0:128], wv_ext[:, :, 0:128])
            nc.gpsimd.dma_start(wv_sb[:, :, 128:DG], wv_ext[:, :, 128:DG])
            for ch in range(1, NQT):
                csl = slice(ch * QT, (ch + 1) * QT)
                nc.sync.dma_start(xt[:, 0:4, csl], xv[:, 0:4, csl])
                nc.scalar.dma_start(xt[:, 4:8, csl], xv[:, 4:8, csl])
            nc.gpsimd.dma_start(wo_sb[:, :, :], wo_ext.ap())
            nc.gpsimd.dma_start(idm[:, :], idm_ext.ap())

            nc.vector.memset(ones2[:, :, :], 1.0)
            # HAM heater: ~4us of dummy matmuls on setup tiles so the PE
            # clock gate is already 8/8 when the first projection lands
            # (first ~3.4us of sustained PE activity runs at 1.2 GHz
            # otherwise).  Results land in a rotating psM slot, never read.
            hps = psM.tile([128, QT], F32, tag="mm_ps")
            for w in range(40):
                nc.tensor.matmul(hps[0:128, 0:128], ones2[:, 0, :],
                                 ones2[:, 0, 0:128],
                                 start=(w == 0), stop=(w == 39))
            nc.vector.memset(qts[64:128, :, :], 0.0)
            nc.vector.memset(kts[64:128, :, :], 0.0)
            nc.vector.tensor_copy(ones2[0:1, 0, 0:1], hps[0:1, 0:1])
            nc.vector.memset(ones2[0:1, 0, 0:1], 1.0)
            nc.gpsimd.affine_select(
                mask2[:, :, :], ones2[:, :, :], pattern=[[0, 2], [1, 128]],
                compare_op=mybir.AluOpType.is_ge, fill=0.0,
                base=0, channel_multiplier=-1)
            nc.vector.memset(hmat[:, :], 0.0)
            nc.vector.memset(hmat[0:64, 0:32], 1.0)    # cols 0..31: even head
            nc.vector.memset(hmat[64:128, 32:33], 1.0)  # col 32: odd head
            nc.vector.memset(sel33[:, :], 0.0)
            nc.vector.memset(sel33[0:1, 0:64], 1.0)
            nc.vector.memset(sel33[32:33, 64:128], 1.0)

            # (wsb, dst, sqscale): rsqrt(n/64) = 8/||q||, rsqrt(n) =
            # 1/||k||; rsqrt(s*x) = exp(-0.5*ln(s*x)) -- same ACT table as Exp
            QK = ((wq_sb, qts, 1.0 / 64.0), (wk_sb, kts, 1.0))

            def qkv_proj(t):
                """Projection matmuls only; the norm chains (DVE/ACT/GPSIMD)
                run behind them while the PE moves on."""
                tsl = slice(t * QT, (t + 1) * QT)
                qsbs = {}
                for qk, (wsb, dst, sqscale) in enumerate(QK):
                    for dc in range(2):
                        pps = psM.tile([128, QT], F32, tag="mm_ps")
                        for kc in range(NKC):
                            nc.tensor.matmul(
                                pps[:, :],
                                wsb[:, kc, dc * 128:(dc + 1) * 128],
                                xt[:, kc, tsl],
                                start=(kc == 0), stop=(kc == NKC - 1))
                        qsb = st.tile([128, QT], F32, tag="qsb", bufs=4)
                        nc.vector.tensor_copy(qsb[:, :], pps[:, :])
                        sq = st.tile([128, QT], BF16, tag="sq", bufs=4)
                        nc.gpsimd.tensor_mul(sq[:, :], qsb[:, :], qsb[:, :])
                        qsbs[(qk, dc)] = (qsb, sq)
                return qsbs

            def qkv_norms(t, qsbs):
                """Norm matmuls, rsqrt batch, broadcast matmuls, V — ordered
                so each PE instruction's inputs are ready when it issues."""
                tsl = slice(t * QT, (t + 1) * QT)

                def v_chunk(tcc):
                    vps = psM.tile([128, QT], F32, tag="mm_ps")
                    for kc in range(NKC):
                        nc.tensor.matmul(
                            vps[:, 0:DG],
                            xt[:, kc, tcc * 128:(tcc + 1) * 128],
                            wv_sb[:, kc, :],
                            start=(kc == 0), stop=(kc == NKC - 1))
                    nc.vector.tensor_copy(
                        vt[:, tcc, :, 0:64],
                        vps[:, 0:DG].rearrange("p (h d) -> p h d", d=64))
                    nc.vector.memset(vt[:, tcc, :, 64:65], 1.0)

                # per-head norm^2 -> rows 0..31 even / row 32 odd of an
                # s2-pool slot; V chunks fill the PE while the ACT ln/exp
                # batch drains nps slots.  dc0 first so attention pair 0 can
                # start as soon as possible.
                npss = {}
                for qk, dc in ((0, 0), (1, 0)):
                    nps = psS.tile([128, 2, QT], F32, tag="s2")
                    nc.tensor.matmul(nps[0:33, 0, :], hmat[:, :],
                                     qsbs[(qk, dc)][1][:, :],
                                     start=True, stop=True)
                    npss[(qk, dc)] = nps
                v_chunk(4 * t)
                v_chunk(4 * t + 1)
                for qk, dc in ((0, 1), (1, 1)):
                    nps = psS.tile([128, 2, QT], F32, tag="s2")
                    nc.tensor.matmul(nps[0:33, 0, :], hmat[:, :],
                                     qsbs[(qk, dc)][1][:, :],
                                     start=True, stop=True)
                    npss[(qk, dc)] = nps
                rdcs = {}
                for qk, dc in ((0, 0), (1, 0), (0, 1), (1, 1)):
                    nps = npss[(qk, dc)]
                    nc.scalar.activation(nps[0:33, 1, :], nps[0:33, 0, :],
                                         AF.Ln, scale=QK[qk][2])
                    rdc = st.tile([33, QT], BF16, tag=f"rdc{qk}{dc}", bufs=2)
                    nc.scalar.activation(rdc[:, :], nps[0:33, 1, :],
                                         AF.Exp, scale=-0.5)
                    rdcs[(qk, dc)] = rdc

                def bc_mul(qk, dc):
                    wsb, dst, sqscale = QK[qk]
                    bc_ps = psM.tile([128, QT], F32, tag="mm_ps")
                    nc.tensor.matmul(bc_ps[:, :], sel33[:, :],
                                     rdcs[(qk, dc)][:, :],
                                     start=True, stop=True)
                    qsb = qsbs[(qk, dc)][0]
                    for half in range(2):
                        pr = 64 * half
                        nc.vector.tensor_mul(
                            dst[0:64, 2 * dc + half, tsl],
                            qsb[pr:pr + 64, :], bc_ps[pr:pr + 64, :])

                # dc0 scales first so attention pair 0's S matmuls find
                # qts/kts ready while V2/V3 still stream on the PE
                bc_mul(0, 0)
                bc_mul(1, 0)
                v_chunk(4 * t + 2)
                v_chunk(4 * t + 3)
                bc_mul(0, 1)
                bc_mul(1, 1)

            def attn_tile(t, fill_work, last=False):
                njc = 4 * (t + 1)

                def s_exp(p, jc):
                    """S matmuls + exp (+ diagonal mask) for one (pair, jc)
                    block; returns the a2 tile for the AV step."""
                    h0, h1 = 2 * p, 2 * p + 1
                    jsl = slice(jc * 128, (jc + 1) * 128)
                    off = 128 * (jc - 4 * t) if jc >= 4 * t else 0
                    s2 = psS.tile([128, 2, QT], F32, tag="s2")
                    for hh, h in ((0, h0), (1, h1)):
                        nc.tensor.matmul(
                            s2[:, hh, off:], kts[:, h, jsl],
                            qts[:, h, t * QT + off:(t + 1) * QT],
                            start=True, stop=True)
                    a2 = asb.tile([128, 2, QT], BF16, tag="a2", bufs=24)
                    nc.scalar.activation(a2[:, :, off:], s2[:, :, off:],
                                         AF.Exp)
                    if jc >= 4 * t:         # diagonal block: mask 128 cols
                        nc.gpsimd.tensor_mul(
                            a2[:, :, off:off + 128],
                            a2[:, :, off:off + 128], mask2[:, :, :])
                    return a2

                def av_group(oacc, p, lic, a2s):
                    # flipped AV: A block is the stationary operand (full
                    # 128x128, FWL-eligible), [V|1] streams 65 columns.
                    # oacc[hh][:, lic, :] accumulates [tok, d|sum].
                    h0 = 2 * p
                    for hh in range(2):
                        last = 4 * t + lic
                        for jc in range(last + 1):
                            nc.tensor.matmul(
                                oacc[hh][:, lic, :],
                                a2s[jc][:, hh, lic * 128:(lic + 1) * 128],
                                vt[:, jc, h0 + hh, :],
                                start=(jc == 0), stop=(jc == last))

                def normalize_lic(oacc, p, lic, pe_tr=False):
                    # per (i-chunk, head): drain PSUM, 1/sum from the ones
                    # column (per-partition scalar), scale, transpose to O^T
                    # on the DMA XBAR (PE transpose for the last tile, where
                    # the ~1.2us XBAR latency would sit on the critical tail)
                    onrm = st.tile([128, 2, DH], BF16, tag="onrm", bufs=10)
                    for hh in range(2):
                        osb = st.tile([128, DH + 1], F32, tag="osb65",
                                      bufs=4)
                        nc.vector.tensor_copy(osb[:, :], oacc[hh][:, lic, :])
                        rs1 = st.tile([128, 1], F32, tag="rs1", bufs=4)
                        nc.vector.reciprocal_approx_fast(
                            out=rs1[:, :], in_=osb[:, 64:65])
                        nc.vector.tensor_scalar_mul(
                            onrm[:, hh, :], osb[:, 0:DH], rs1[:, :])
                    csl = slice(t * QT + lic * 128, t * QT + (lic + 1) * 128)
                    if pe_tr:
                        tp = psM.tile([128, QT], F32, tag="mm_ps")
                        tpv = tp[:, 0:64].bitcast(BF16)
                        nc.tensor.transpose(tpv, onrm[:, :, :], idm[:, :])
                        nc.vector.tensor_copy(ot[:, p, csl], tpv)
                    else:
                        nc.sync.dma_start_transpose(ot[:, p, csl],
                                                    onrm[:, :, :])

                # Phase structure: all S/exp blocks of a pair are buffered
                # (a2 tiles), then AV runs region-major.  The exp-paced S
                # phases are interleaved with dense matmul work (outproj for
                # pair0, pair0's AV for pair1) to keep the PE busy.
                oacc0 = [psO.tile([128, 4, DH + 1], F32, tag=f"oacc{hh}",
                                  name=f"oacc_{t}_0_{hh}")
                         for hh in range(2)]
                jco = list(range(njc))
                a2s0 = {}
                for jc in jco:
                    a2s0[jc] = s_exp(0, jc)
                    if fill_work:
                        fill_work.pop(0)()
                while fill_work:
                    fill_work.pop(0)()
                oacc1 = [psO.tile([128, 4, DH + 1], F32, tag=f"oacc{hh}",
                                  name=f"oacc_{t}_1_{hh}")
                         for hh in range(2)]
                a2s1 = {}
                avq = [0, 1, 2, 3]
                for i, jc in enumerate(jco):
                    a2s1[jc] = s_exp(1, jc)
                    if i % 2 == 1 and avq:
                        lic = avq.pop(0)
                        av_group(oacc0, 0, lic, a2s0)
                        normalize_lic(oacc0, 0, lic)
                for lic in avq:
                    av_group(oacc0, 0, lic, a2s0)
                    normalize_lic(oacc0, 0, lic)
                for lic in range(4):
                    av_group(oacc1, 1, lic, a2s1)
                    normalize_lic(oacc1, 1, lic, pe_tr=last)

            def outproj_chunk(t, mc):
                    tsl = slice(t * QT, (t + 1) * QT)
                    ops = psM.tile([128, QT], F32, tag="mm_ps")
                    for dc in range(2):
                        nc.tensor.matmul(
                            ops[:, :],
                            wo_sb[:, dc, mc * 128:(mc + 1) * 128],
                            ot[:, dc, tsl],
                            start=(dc == 0), stop=(dc == 1))
                    osb = st.tile([128, QT], BF16, tag="osb", bufs=8)
                    nc.vector.tensor_copy(osb[:, :], ops[:, :])
                    nc.sync.dma_start(out_ext[mc * 128:(mc + 1) * 128, tsl],
                                      osb[:, :])

            def outproj_thunks(t):
                return [(lambda mc=mc: outproj_chunk(t, mc))
                        for mc in range(MOUT // 128)]

            # Software pipeline: attention lags the projections by one tile
            # so the tile-i norm chain (ACT ln/exp) never queues behind the
            # attn(i-1) exp flood, and attn always finds qts/kts/vt ready.
            qsbs = qkv_proj(0)
            qkv_norms(0, qsbs)
            for t in range(1, NQT):
                qsbs = qkv_proj(t)
                qkv_norms(t, qsbs)
                attn_tile(t - 1, outproj_thunks(t - 2) if t >= 2 else [])
            attn_tile(NQT - 1, outproj_thunks(NQT - 2), last=True)

            # final out-projection, software-pipelined two mc-chunks wide:
            # the dc0 matmuls depend only on pair 0 of the last tile, so
            # they fill the PE while pair 1's normalize chain drains; the
            # psum copies alternate vector/scalar (scalar is idle by now)
            tl = slice((NQT - 1) * QT, NQT * QT)
            opp = None
            for mc in range(MOUT // 128):
                ops = psM.tile([128, QT], F32, tag="mm_ps")
                nc.tensor.matmul(ops[:, :],
                                 wo_sb[:, 0, mc * 128:(mc + 1) * 128],
                                 ot[:, 0, tl], start=True, stop=False)
                if opp is not None:
                    pmc, pops = opp
                    nc.tensor.matmul(pops[:, :],
                                     wo_sb[:, 1, pmc * 128:(pmc + 1) * 128],
                                     ot[:, 1, tl], start=False, stop=True)
                    osb = st.tile([128, QT], BF16, tag="osb", bufs=8)
                    nc.vector.tensor_copy(osb[:, :], pops[:, :])
                    nc.sync.dma_start(
                        out_ext[pmc * 128:(pmc + 1) * 128, tl], osb[:, :])
                opp = (mc, ops)
            pmc, pops = opp
            nc.tensor.matmul(pops[:, :],
                             wo_sb[:, 1, pmc * 128:(pmc + 1) * 128],
                             ot[:, 1, tl], start=False, stop=True)
            osb = st.tile([128, QT], BF16, tag="osb", bufs=8)
            nc.vector.tensor_copy(osb[:, :], pops[:, :])
            nc.sync.dma_start(out_ext[pmc * 128:(pmc + 1) * 128, tl],
                              osb[:, :])

    nc.compile()
    return nc


_NC_CACHE = {}


def _get_nc(N=N_TOK):
    if N not in _NC_CACHE:
        _NC_CACHE[N] = build_nc(N)
    return _NC_CACHE[N]


def _marshal_w(w):
    """[c*128, n] -> [128, c, n] bf16 contiguous (device SBUF layout)."""
    c = w.shape[0] // 128
    return np.ascontiguousarray(
        w.reshape(c, 128, -1).transpose(1, 0, 2)).astype(mybir.dt.np(BF16))


def make_in_maps(x, Wq, Wk, Wv, Wo):
    idm = np.eye(128, dtype=mybir.dt.np(BF16))
    in_maps = []
    for c in range(8):
        b, g = divmod(c, 2)
        gsl = slice(g * DG, (g + 1) * DG)
        in_maps.append({
            "xt": np.ascontiguousarray(x[b].T).astype(mybir.dt.np(BF16)),
            "wq": _marshal_w(Wq[:, gsl]),
            "wk": _marshal_w(Wk[:, gsl]),
            "wv": _marshal_w(Wv[:, gsl]),
            "wo": _marshal_w(Wo[gsl, :]),
            "idm": idm,
        })
    return in_maps


def kernel(x, Wq, Wk, Wv, Wo, _trace=False):
    x = np.asarray(x)
    nc = _get_nc(x.shape[1])
    in_maps = make_in_maps(np.asarray(x), np.asarray(Wq), np.asarray(Wk),
                           np.asarray(Wv), np.asarray(Wo))
    res = run_bass_kernel_spmd(nc, in_maps, core_ids=list(range(8)),
                               trace=_trace)
    kernel.last_results = res
    out = np.empty((x.shape[0], x.shape[1], MOUT), dtype=np.float32)
    for b in range(x.shape[0]):
        a = res.results[2 * b]["out"].astype(np.float32)
        c = res.results[2 * b + 1]["out"].astype(np.float32)
        out[b] = (a + c).T
    return out


# revision 26
# speedup vs baseline: 1.2028x; 1.2028x over previous
"""Trainium2 Bass kernel for causal cosine-sim multi-head attention.

Reference computation (per batch b of 4, 2048 tokens, dim 1024):
  q,k,v = x @ Wq, x @ Wk, x @ Wv          (inner = 8 heads x 64)
  q,k l2-normalized per head, scale 8.0, causal softmax, out = attn-out @ Wo

Sharding: 8 cores = 4 batches x 2 head-groups (4 heads each).
Core c handles batch c//2, heads [4*(c%2), 4*(c%2)+4).  Each core computes a
partial output (2048, 1024) = o_g @ Wo_g; host sums the two head-group
partials per batch.  No on-chip collectives; the 8 cores run SPMD.

Schedule notes (final):
- Software pipeline with attention lagging one i-tile behind the
  projections: iteration i runs qkv_proj(i) + norms(i) on a dense PE
  stream, then attn(i-1).  The tile-i rsqrt (ACT) therefore only competes
  with the long-drained exp backlog of attn(i-2), never stalling the
  norm->bc->S chain, and the PE never sees an ACT-FIFO bubble.
- qts/kts are 128-partition with zero rows 64..127: the S stationary is
  128-deep, so FWL hides its LDWEIGHTS (64-deep stationaries measured
  +156 ns/matmul of exposed weight-load; this was worth ~20 us).
- All ACT work (softmax exp + the l2-norm rsqrt = exp(-0.5*ln(x))) lives in
  the single natural_log_exp table set: the activation-table map is patched
  (compile-time only, index-preserving) so the table pass can only pick that
  set -> zero ACT_TABLE_LOADs in steady state.
- A ~4us dummy-matmul "heater" at kernel start pre-warms the PE_HAM clock
  gate (K=4/8 -> 8/8) under the input-DMA window.
- O^T transpose runs on the DMA XBAR (dma_start_transpose), not the PE, so
  the matmul queue never stalls behind the DVE normalize chain; the last
  tile uses the PE/identity transpose instead, keeping the ~1.2us XBAR
  latency off the critical tail.
- GpSimd (otherwise idle) takes only same-type TensorTensor multiplies (the
  causal mask apply + the q/k squares) -> no Q7 library reloads, and no
  cross-iteration Pool-FIFO priority inversions.
- Diagonal S/exp blocks are issued first within each pair (jco order) so
  their mask multiplies are ready before the AV accumulation phase.
- Softmax 1/sum comes free from the [V | 1] ones column in the AV matmul.
- Causal diagonal blocks compute only the needed column ranges in S, exp, AV.
"""

import numpy as np

import concourse.bass as bass
import concourse.bacc as bacc
import concourse.mybir as mybir
import concourse.tile as tile
import concourse.hw_specs as _hw_specs
from concourse.bass_utils import run_bass_kernel_spmd

DT = mybir.dt
F32 = DT.float32
BF16 = DT.bfloat16

N_TOK = 2048
DIM = 1024
DG = 256          # inner dims per core (4 heads x 64)
NH = 4            # heads per core
DH = 64
MOUT = 1024

_KEEP_TABLE = "natural_log_exp_and_others"


def _patch_act_tables():
    """Make the compile-time table pass see only one usable ACT table set
    (ln+exp+copy+...), so Ln and Exp never thrash table loads.  Names/indices
    are preserved (act_func_set_id is positional); only the advertised
    contents change, and the kernel only uses funcs the real set contains."""
    import functools

    orig = _hw_specs.get_activation_tables
    if getattr(orig, "_one_set_patch", False):
        return
    inner = getattr(orig, "__wrapped__", orig)

    @functools.cache
    def patched(arch):
        t = inner(arch)
        assert _KEEP_TABLE in t, sorted(t)
        return {name: (set(funcs) if name == _KEEP_TABLE else set())
                for name, funcs in t.items()}

    patched._one_set_patch = True
    _hw_specs.get_activation_tables = patched
    bacc.get_activation_tables = patched


def build_nc(N=N_TOK):
    _patch_act_tables()
    NKC = DIM // 128          # 8 contraction chunks
    NTC = N // 128            # token chunks
    QT = 512                  # token tile (qkv projection and attention i)
    NQT = N // QT
    AF = mybir.ActivationFunctionType

    nc = bacc.Bacc("TRN2", target_bir_lowering=False, debug=False, num_devices=8)
    xt_ext = nc.dram_tensor("xt", [DIM, N], BF16, kind="ExternalInput")
    wq_ext = nc.dram_tensor("wq", [128, NKC, DG], BF16, kind="ExternalInput")
    wk_ext = nc.dram_tensor("wk", [128, NKC, DG], BF16, kind="ExternalInput")
    wv_ext = nc.dram_tensor("wv", [128, NKC, DG], BF16, kind="ExternalInput")
    wo_ext = nc.dram_tensor("wo", [128, DG // 128, MOUT], BF16,
                            kind="ExternalInput")
    idm_ext = nc.dram_tensor("idm", [128, 128], BF16, kind="ExternalInput")
    out_ext = nc.dram_tensor("out", [MOUT, N], BF16, kind="ExternalOutput")

    with tile.TileContext(nc) as tc:
        with (
            tc.tile_pool(name="persist", bufs=1) as pp,
            tc.tile_pool(name="stage", bufs=3) as st,
            tc.tile_pool(name="attn_sb", bufs=4) as asb,
            tc.tile_pool(name="ps_mm", bufs=2, space="PSUM") as psM,
            tc.tile_pool(name="ps_s", bufs=2, space="PSUM") as psS,
            tc.tile_pool(name="ps_o", bufs=1, space="PSUM") as psO,
        ):
            xt = pp.tile([128, NKC, N], BF16, tag="xt")          # x transposed
            wq_sb = pp.tile([128, NKC, DG], BF16, tag="wq")
            wk_sb = pp.tile([128, NKC, DG], BF16, tag="wk")
            wv_sb = pp.tile([128, NKC, DG], BF16, tag="wv")
            wo_sb = pp.tile([128, 2, MOUT], BF16, tag="wo")
            # 128-partition q/k with zero rows 64-127: the S stationary is
            # then 128-deep -> FWL hides its LDWEIGHTS (64-deep stationaries
            # measured +156ns/matmul of exposed weight-load)
            qts = pp.tile([128, NH, N], BF16, tag="qts")         # scaled Q^T
            kts = pp.tile([128, NH, N], BF16, tag="kts")         # scaled K^T
            vt = pp.tile([128, NTC, NH, DH + 1], BF16, tag="vt")  # [V | 1]
            ot = pp.tile([128, 2, N], BF16, tag="ot")            # normed O^T
            hmat = pp.tile([128, 33], BF16, tag="hmat")  # head-sum matrix
            # broadcast selector: out rows 0..63 <- rhs row 0 (even head),
            # rows 64..127 <- rhs row 32 (odd head); one matmul per pair
            sel33 = pp.tile([33, 128], BF16, tag="sel33")
            # causal mask for the partial 128 columns of a diagonal block,
            # duplicated for the 2 heads of a pair: keep where col >= row
            mask2 = pp.tile([128, 2, 128], BF16, tag="mask2")
            ones2 = pp.tile([128, 2, 128], BF16, tag="ones2")
            idm = pp.tile([128, 128], BF16, tag="idm")   # identity (transpose)

            # ---- input DMAs first.  First Q-projection gates on wq-dc0 + the
            # 8 k-chunks of tile 0, so those go first, tile-0 chunks spread
            # across all three DMA issue queues (sync/scalar/gpsimd) ----
            xv = xt_ext.rearrange("(c p) n -> p c n", p=128)
            # per-queue, ordered by first PE use: wq-dc0, x-tile0, wq-dc1,
            # wk (K-proj at ~PE+2us), wv (V at ~PE+6us), x tiles 1-3, wo
            nc.gpsimd.dma_start(wq_sb[:, :, 0:128], wq_ext[:, :, 0:128])
            t0q = (nc.sync, nc.scalar, nc.gpsimd)
            for kc in range(NKC):
                t0q[kc % 3].dma_start(xt[:, kc:kc + 1, 0:256],
                                      xv[:, kc:kc + 1, 0:256])
            for kc in range(NKC):
                t0q[kc % 3].dma_start(xt[:, kc:kc + 1, 256:QT],
                                      xv[:, kc:kc + 1, 256:QT])
            nc.gpsimd.dma_start(wq_sb[:, :, 128:DG], wq_ext[:, :, 128:DG])
            nc.gpsimd.dma_start(wk_sb[:, :, 0:128], wk_ext[:, :, 0:128])
            nc.sync.dma_start(wk_sb[:, :, 128:DG], wk_ext[:, :, 128:DG])
            nc.scalar.dma_start(wv_sb[:, :, 0:128], wv_ext[:, :, 0:128])
            nc.gpsimd.dma_start(wv_sb[:, :, 128:DG], wv_ext[:, :, 128:DG])
            for ch in range(1, NQT):
                csl = slice(ch * QT, (ch + 1) * QT)
                nc.sync.dma_start(xt[:, 0:4, csl], xv[:, 0:4, csl])
                nc.scalar.dma_start(xt[:, 4:8, csl], xv[:, 4:8, csl])
            nc.gpsimd.dma_start(wo_sb[:, :, :], wo_ext.ap())
            nc.gpsimd.dma_start(idm[:, :], idm_ext.ap())

            nc.vector.memset(ones2[:, :, :], 1.0)
            # HAM heater: ~4us of dummy matmuls on setup tiles so the PE
            # clock gate is already 8/8 when the first projection lands
            # (first ~3.4us of sustained PE activity runs at 1.2 GHz
            # otherwise).  Results land in a rotating psM slot, never read.
            hps = psM.tile([128, QT], F32, tag="mm_ps")
            for w in range(40):
                nc.tensor.matmul(hps[0:128, 0:128], ones2[:, 0, :],
                                 ones2[:, 0, 0:128],
                                 start=(w == 0), stop=(w == 39))
            nc.vector.memset(qts[64:128, :, :], 0.0)
            nc.vector.memset(kts[64:128, :, :], 0.0)
            nc.vector.tensor_copy(ones2[0:1, 0, 0:1], hps[0:1, 0:1])
            nc.vector.memset(ones2[0:1, 0, 0:1], 1.0)
            nc.gpsimd.affine_select(
                mask2[:, :, :], ones2[:, :, :], pattern=[[0, 2], [1, 128]],
                compare_op=mybir.AluOpType.is_ge, fill=0.0,
                base=0, channel_multiplier=-1)
            nc.vector.memset(hmat[:, :], 0.0)
            nc.vector.memset(hmat[0:64, 0:32], 1.0)    # cols 0..31: even head
            nc.vector.memset(hmat[64:128, 32:33], 1.0)  # col 32: odd head
            nc.vector.memset(sel33[:, :], 0.0)
            nc.vector.memset(sel33[0:1, 0:64], 1.0)
            nc.vector.memset(sel33[32:33, 64:128], 1.0)

            # (wsb, dst, sqscale): rsqrt(n/64) = 8/||q||, rsqrt(n) =
            # 1/||k||; rsqrt(s*x) = exp(-0.5*ln(s*x)) -- same ACT table as Exp
            QK = ((wq_sb, qts, 1.0 / 64.0), (wk_sb, kts, 1.0))

            def qkv_proj(t):
                """Projection matmuls only; the norm chains (DVE/ACT/GPSIMD)
                run behind them while the PE moves on."""
                tsl = slice(t * QT, (t + 1) * QT)
                qsbs = {}
                # t=0 runs per-256-col halves so the first matmuls gate on
                # half-landed x DMAs (startup latency)
                halves = ((slice(t * QT, t * QT + 256),
                           slice(t * QT + 256, (t + 1) * QT))
                          if t == 0 else (tsl,))
                for qk, (wsb, dst, sqscale) in enumerate(QK):
                    for dc in range(2):
                        pps = psM.tile([128, QT], F32, tag="mm_ps")
                        for hi, hsl in enumerate(halves):
                            po = (hsl.start - t * QT)
                            pw = hsl.stop - hsl.start
                            for kc in range(NKC):
                                nc.tensor.matmul(
                                    pps[:, po:po + pw],
                                    wsb[:, kc, dc * 128:(dc + 1) * 128],
                                    xt[:, kc, hsl],
                                    start=(kc == 0), stop=(kc == NKC - 1))
                        qsb = st.tile([128, QT], F32, tag="qsb", bufs=4)
                        nc.vector.tensor_copy(qsb[:, :], pps[:, :])
                        sq = st.tile([128, QT], BF16, tag="sq", bufs=4)
                        nc.gpsimd.tensor_mul(sq[:, :], qsb[:, :], qsb[:, :])
                        qsbs[(qk, dc)] = (qsb, sq)
                return qsbs

            def qkv_norms(t, qsbs):
                """Norm matmuls, rsqrt batch, broadcast matmuls, V — ordered
                so each PE instruction's inputs are ready when it issues."""
                tsl = slice(t * QT, (t + 1) * QT)

                def v_chunk(tcc):
                    vps = psM.tile([128, QT], F32, tag="mm_ps")
                    for kc in range(NKC):
                        nc.tensor.matmul(
                            vps[:, 0:DG],
                            xt[:, kc, tcc * 128:(tcc + 1) * 128],
                            wv_sb[:, kc, :],
                            start=(kc == 0), stop=(kc == NKC - 1))
                    nc.vector.tensor_copy(
                        vt[:, tcc, :, 0:64],
                        vps[:, 0:DG].rearrange("p (h d) -> p h d", d=64))
                    nc.vector.memset(vt[:, tcc, :, 64:65], 1.0)

                # per-head norm^2 -> rows 0..31 even / row 32 odd of an
                # s2-pool slot; V chunks fill the PE while the ACT ln/exp
                # batch drains nps slots.  dc0 first so attention pair 0 can
                # start as soon as possible.
                npss = {}
                for qk, dc in ((0, 0), (1, 0)):
                    nps = psS.tile([128, 2, QT], F32, tag="s2")
                    nc.tensor.matmul(nps[0:33, 0, :], hmat[:, :],
                                     qsbs[(qk, dc)][1][:, :],
                                     start=True, stop=True)
                    npss[(qk, dc)] = nps
                v_chunk(4 * t)
                v_chunk(4 * t + 1)
                for qk, dc in ((0, 1), (1, 1)):
                    nps = psS.tile([128, 2, QT], F32, tag="s2")
                    nc.tensor.matmul(nps[0:33, 0, :], hmat[:, :],
                                     qsbs[(qk, dc)][1][:, :],
                                     start=True, stop=True)
                    npss[(qk, dc)] = nps
                rdcs = {}
                for qk, dc in ((0, 0), (1, 0), (0, 1), (1, 1)):
                    nps = npss[(qk, dc)]
                    nc.scalar.activation(nps[0:33, 1, :], nps[0:33, 0, :],
                                         AF.Ln, scale=QK[qk][2])
                    rdc = st.tile([33, QT], BF16, tag=f"rdc{qk}{dc}", bufs=2)
                    nc.scalar.activation(rdc[:, :], nps[0:33, 1, :],
                                         AF.Exp, scale=-0.5)
                    rdcs[(qk, dc)] = rdc

                def bc_mul(qk, dc):
                    wsb, dst, sqscale = QK[qk]
                    bc_ps = psM.tile([128, QT], F32, tag="mm_ps")
                    nc.tensor.matmul(bc_ps[:, :], sel33[:, :],
                                     rdcs[(qk, dc)][:, :],
                                     start=True, stop=True)
                    qsb = qsbs[(qk, dc)][0]
                    for half in range(2):
                        pr = 64 * half
                        nc.vector.tensor_mul(
                            dst[0:64, 2 * dc + half, tsl],
                            qsb[pr:pr + 64, :], bc_ps[pr:pr + 64, :])

                # dc0 scales first so attention pair 0's S matmuls find
                # qts/kts ready while V2/V3 still stream on the PE
                bc_mul(0, 0)
                bc_mul(1, 0)
                v_chunk(4 * t + 2)
                v_chunk(4 * t + 3)
                bc_mul(0, 1)
                bc_mul(1, 1)

            def attn_tile(t, fill_work, last=False):
                njc = 4 * (t + 1)

                def s_exp(p, jc):
                    """S matmuls + exp (+ diagonal mask) for one (pair, jc)
                    block; returns the a2 tile for the AV step."""
                    h0, h1 = 2 * p, 2 * p + 1
                    jsl = slice(jc * 128, (jc + 1) * 128)
                    off = 128 * (jc - 4 * t) if jc >= 4 * t else 0
                    s2 = psS.tile([128, 2, QT], F32, tag="s2")
                    for hh, h in ((0, h0), (1, h1)):
                        nc.tensor.matmul(
                            s2[:, hh, off:], kts[:, h, jsl],
                            qts[:, h, t * QT + off:(t + 1) * QT],
                            start=True, stop=True)
                    a2 = asb.tile([128, 2, QT], BF16, tag="a2", bufs=24)
                    nc.scalar.activation(a2[:, :, off:], s2[:, :, off:],
                                         AF.Exp)
                    if jc >= 4 * t:         # diagonal block: mask 128 cols
                        nc.gpsimd.tensor_mul(
                            a2[:, :, off:off + 128],
                            a2[:, :, off:off + 128], mask2[:, :, :])
                    return a2

                def av_group(oacc, p, lic, a2s):
                    # flipped AV: A block is the stationary operand (full
                    # 128x128, FWL-eligible), [V|1] streams 65 columns.
                    # oacc[hh][:, lic, :] accumulates [tok, d|sum].
                    h0 = 2 * p
                    for hh in range(2):
                        last = 4 * t + lic
                        for jc in range(last + 1):
                            nc.tensor.matmul(
                                oacc[hh][:, lic, :],
                                a2s[jc][:, hh, lic * 128:(lic + 1) * 128],
                                vt[:, jc, h0 + hh, :],
                                start=(jc == 0), stop=(jc == last))

                def normalize_lic(oacc, p, lic, pe_tr=False):
                    # per (i-chunk, head): drain PSUM, 1/sum from the ones
                    # column (per-partition scalar), scale, transpose to O^T
                    # on the DMA XBAR (PE transpose for the last tile, where
                    # the ~1.2us XBAR latency would sit on the critical tail)
                    onrm = st.tile([128, 2, DH], BF16, tag="onrm", bufs=10)
                    for hh in range(2):
                        osb = st.tile([128, DH + 1], F32, tag="osb65",
                                      bufs=4)
                        nc.vector.tensor_copy(osb[:, :], oacc[hh][:, lic, :])
                        rs1 = st.tile([128, 1], F32, tag="rs1", bufs=4)
                        nc.vector.reciprocal_approx_fast(
                            out=rs1[:, :], in_=osb[:, 64:65])
                        nc.vector.tensor_scalar_mul(
                            onrm[:, hh, :], osb[:, 0:DH], rs1[:, :])
                    csl = slice(t * QT + lic * 128, t * QT + (lic + 1) * 128)
                    if pe_tr:
                        tp = psM.tile([128, QT], F32, tag="mm_ps")
                        tpv = tp[:, 0:64].bitcast(BF16)
                        nc.tensor.transpose(tpv, onrm[:, :, :], idm[:, :])
                        nc.vector.tensor_copy(ot[:, p, csl], tpv)
                    else:
                        nc.sync.dma_start_transpose(ot[:, p, csl],
                                                    onrm[:, :, :])

                # Phase structure: all S/exp blocks of a pair are buffered
                # (a2 tiles), then AV runs region-major.  The exp-paced S
                # phases are interleaved with dense matmul work (outproj for
                # pair0, pair0's AV for pair1) to keep the PE busy.
                oacc0 = [psO.tile([128, 4, DH + 1], F32, tag=f"oacc{hh}",
                                  name=f"oacc_{t}_0_{hh}")
                         for hh in range(2)]
                jco = list(range(njc))
                a2s0 = {}
                for jc in jco:
                    a2s0[jc] = s_exp(0, jc)
                    if fill_work:
                        fill_work.pop(0)()
                while fill_work:
                    fill_work.pop(0)()
                oacc1 = [psO.tile([128, 4, DH + 1], F32, tag=f"oacc{hh}",
                                  name=f"oacc_{t}_1_{hh}")
                         for hh in range(2)]
                a2s1 = {}
                avq = [0, 1, 2, 3]
                for i, jc in enumerate(jco):
                    a2s1[jc] = s_exp(1, jc)
                    if i % 2 == 1 and avq:
                        lic = avq.pop(0)
                        av_group(oacc0, 0, lic, a2s0)
                        normalize_lic(oacc0, 0, lic)
                for lic in avq:
                    av_group(oacc0, 0, lic, a2s0)
                    normalize_lic(oacc0, 0, lic)
                for lic in range(4):
                    av_group(oacc1, 1, lic, a2s1)
                    normalize_lic(oacc1, 1, lic, pe_tr=last)

            def outproj_chunk(t, mc):
                    tsl = slice(t * QT, (t + 1) * QT)
                    ops = psM.tile([128, QT], F32, tag="mm_ps")
                    for dc in range(2):
                        nc.tensor.matmul(
                            ops[:, :],
                            wo_sb[:, dc, mc * 128:(mc + 1) * 128],
                            ot[:, dc, tsl],
                            start=(dc == 0), stop=(dc == 1))
                    osb = st.tile([128, QT], BF16, tag="osb", bufs=8)
                    nc.vector.tensor_copy(osb[:, :], ops[:, :])
                    nc.sync.dma_start(out_ext[mc * 128:(mc + 1) * 128, tsl],
                                      osb[:, :])

            def outproj_thunks(t):
                return [(lambda mc=mc: outproj_chunk(t, mc))
                        for mc in range(MOUT // 128)]

            # Software pipeline: attention lags the projections by one tile
            # so the tile-i norm chain (ACT ln/exp) never queues behind the
            # attn(i-1) exp flood, and attn always finds qts/kts/vt ready.
            qsbs = qkv_proj(0)
            qkv_norms(0, qsbs)
            for t in range(1, NQT):
                qsbs = qkv_proj(t)
                qkv_norms(t, qsbs)
                attn_tile(t - 1, outproj_thunks(t - 2) if t >= 2 else [])
            attn_tile(NQT - 1, outproj_thunks(NQT - 2), last=True)

            # final out-projection, software-pipelined two mc-chunks wide:
            # the dc0 matmuls depend only on pair 0 of the last tile, so
            # they fill the PE while pair 1's normalize chain drains; the
            # psum copies alternate vector/scalar (scalar is idle by now)
            tl = slice((NQT - 1) * QT, NQT * QT)
            opp = None
            for mc in range(MOUT // 128):
                ops = psM.tile([128, QT], F32, tag="mm_ps")
                nc.tensor.matmul(ops[:, :],
                                 wo_sb[:, 0, mc * 128:(mc + 1) * 128],
                                 ot[:, 0, tl], start=True, stop=False)
                if opp is not None:
                    pmc, pops = opp
                    nc.tensor.matmul(pops[:, :],
                                     wo_sb[:, 1, pmc * 128:(pmc + 1) * 128],
                                     ot[:, 1, tl], start=False, stop=True)
                    osb = st.tile([128, QT], BF16, tag="osb", bufs=8)
                    nc.vector.tensor_copy(osb[:, :], pops[:, :])
                    nc.sync.dma_start(
                        out_ext[pmc * 128:(pmc + 1) * 128, tl], osb[:, :])
                opp = (mc, ops)
            pmc, pops = opp
            nc.tensor.matmul(pops[:, :],
                             wo_sb[:, 1, pmc * 128:(pmc + 1) * 128],
                             ot[:, 1, tl], start=False, stop=True)
            osb = st.tile([128, QT], BF16, tag="osb", bufs=8)
            nc.vector.tensor_copy(osb[:, :], pops[:, :])
            nc.sync.dma_start(out_ext[pmc * 128:(pmc + 1) * 128, tl],
                              osb[:, :])

    nc.compile()
    return nc


_NC_CACHE = {}


def _get_nc(N=N_TOK):
    if N not in _NC_CACHE:
        _NC_CACHE[N] = build_nc(N)
    return _NC_CACHE[N]


def _marshal_w(w):
    """[c*128, n] -> [128, c, n] bf16 contiguous (device SBUF layout)."""
    c = w.shape[0] // 128
    return np.ascontiguousarray(
        w.reshape(c, 128, -1).transpose(1, 0, 2)).astype(mybir.dt.np(BF16))


def make_in_maps(x, Wq, Wk, Wv, Wo):
    idm = np.eye(128, dtype=mybir.dt.np(BF16))
    in_maps = []
    for c in range(8):
        b, g = divmod(c, 2)
        gsl = slice(g * DG, (g + 1) * DG)
        in_maps.append({
            "xt": np.ascontiguousarray(x[b].T).astype(mybir.dt.np(BF16)),
            "wq": _marshal_w(Wq[:, gsl]),
            "wk": _marshal_w(Wk[:, gsl]),
            "wv": _marshal_w(Wv[:, gsl]),
            "wo": _marshal_w(Wo[gsl, :]),
            "idm": idm,
        })
    return in_maps


def kernel(x, Wq, Wk, Wv, Wo, _trace=False):
    x = np.asarray(x)
    nc = _get_nc(x.shape[1])
    in_maps = make_in_maps(np.asarray(x), np.asarray(Wq), np.asarray(Wk),
                           np.asarray(Wv), np.asarray(Wo))
    res = run_bass_kernel_spmd(nc, in_maps, core_ids=list(range(8)),
                               trace=_trace)
    kernel.last_results = res
    out = np.empty((x.shape[0], x.shape[1], MOUT), dtype=np.float32)
    for b in range(x.shape[0]):
        a = res.results[2 * b]["out"].astype(np.float32)
        c = res.results[2 * b + 1]["out"].astype(np.float32)
        out[b] = (a + c).T
    return out
